# revision 1
# baseline (speedup 1.0000x reference)
"""3x3 neighborhood cosine-similarity sum (minus self) on 8 TRN2 NeuronCores.

Input:  input_image [1024, 1024, 1, 128] float32  (H, W, 1, C)
Output: sim [1024, 1024] float32

Algorithm per pixel: sim = <xn, BoxSum3x3(xn)> - 1, where xn = x / max(||x||, eps).

Sharding: H rows split 128/core across 8 cores; each core gets its 128 rows
plus 1 halo row above/below (zero rows at the image edges), i.e. [130, 1024, 128].

Per-core layout ("layout A"): SBUF tiles [128 part = w%128, free = (j=w//128, c)].
 - ss      : per-chunk fused tensor_tensor_reduce (x*x, add-accum) on DVE
 - inv     : sqrt(ss + 1e-16) on ACT, reciprocal on DVE
 - xn      : per-chunk tensor_scalar mult (f32 in, bf16 out) on DVE
 - vertical: 2 tensor_tensor adds (bf16) on DVE over the xn row ring
 - horizontal: band-matrix matmuls on PE (tridiag within chunk + 2 cross-chunk
   single-entry matrices), accumulated in PSUM
 - evac    : PSUM -> SBUF bf16 copy on ACT
 - dot     : per-chunk fused tensor_tensor_reduce (xn*S, add-accum, init=-1) on DVE
Output rows staged as [blk, p, j*16+rr] and untangled on the host.
"""

import numpy as np
import ml_dtypes

import sys

for _p in ("/opt/trn_rl_repo",):
    if _p not in sys.path:
        sys.path.insert(0, _p)

import concourse.bass as bass
import concourse.bacc as bacc
import concourse.mybir as mybir
import concourse.tile as tile
from concourse.bass_utils import run_bass_kernel_spmd

F32 = mybir.dt.float32
BF16 = mybir.dt.bfloat16
ALU = mybir.AluOpType
ACTF = mybir.ActivationFunctionType

H, W, C = 1024, 1024, 128
NCORES = 8
ROWS_PER_CORE = H // NCORES          # 128
NJ = W // 128                        # 8 w-chunks
RBLK = 16                            # output rows per staging block


def build_consts():
    """Host-side constant matrices for the horizontal box-sum matmuls."""
    t = np.zeros((128, 128), np.float32)
    for k in range(128):
        for m in (k - 1, k, k + 1):
            if 0 <= m < 128:
                t[k, m] = 1.0
    el = np.zeros((128, 128), np.float32)
    el[127, 0] = 1.0
    er = np.zeros((128, 128), np.float32)
    er[0, 127] = 1.0
    to_bf = lambda a: a.astype(ml_dtypes.bfloat16)
    return to_bf(t), to_bf(el), to_bf(er)


def build_bass(n_out_rows=ROWS_PER_CORE):
    """Build the per-core Bass graph. n_out_rows output rows need
    n_out_rows + 2 input rows (zero-padded halo included by the host)."""
    n_in = n_out_rows + 2
    nblk = (n_out_rows + RBLK - 1) // RBLK

    nc = bacc.Bacc(None, target_bir_lowering=False)
    x_dram = nc.declare_dram_parameter("x", [n_in, W, C], F32, isOutput=False)
    band_dram = nc.declare_dram_parameter("band", [128, 128], BF16, isOutput=False)
    el_dram = nc.declare_dram_parameter("el", [128, 128], BF16, isOutput=False)
    er_dram = nc.declare_dram_parameter("er", [128, 128], BF16, isOutput=False)
    out_dram = nc.declare_dram_parameter(
        "out", [nblk, 128, NJ * RBLK], F32, isOutput=True
    )

    with tile.TileContext(nc) as tc:
        with (
            tc.tile_pool(name="consts", bufs=1) as cpool,
            tc.tile_pool(name="xin", bufs=3) as xpool,
            tc.tile_pool(name="sq", bufs=2) as sqpool,
            tc.tile_pool(name="norm", bufs=3) as npool,
            tc.tile_pool(name="xn", bufs=5) as xnpool,
            tc.tile_pool(name="v", bufs=2) as vpool,
            tc.tile_pool(name="sb", bufs=2) as sbpool,
            tc.tile_pool(name="pd", bufs=2) as pdpool,
            tc.tile_pool(name="sim", bufs=2) as simpool,
            tc.tile_pool(name="psum", bufs=2, space="PSUM") as psumpool,
        ):
            band = cpool.tile([128, 128], BF16, tag="band")
            el = cpool.tile([128, 128], BF16, tag="el")
            er = cpool.tile([128, 128], BF16, tag="er")
            nc.sync.dma_start(band[:], band_dram[:])
            nc.sync.dma_start(el[:], el_dram[:])
            nc.sync.dma_start(er[:], er_dram[:])
            eps_bias = cpool.tile([128, 1], F32, tag="eps")
            nc.gpsimd.memset(eps_bias[:], 1e-16)

            xn_rows = [None] * n_in
            simt = None

            for h in range(n_in):
                # ---- load row h: [128 p, NJ j, 128 c], w = j*128 + p
                xt = xpool.tile([128, NJ, C], F32, tag="xt")
                nc.sync.dma_start(
                    xt[:], x_dram[h].rearrange("(j p) c -> p j c", p=128)
                )

                # ---- ss[p, j] = sum_c x^2 (fused mult+reduce per chunk)
                sq = sqpool.tile([128, NJ, C], BF16, tag="sq")
                ssr = npool.tile([128, NJ], F32, tag="ssr")
                import os
                if os.environ.get("SS_POW", "0") == "1":
                    for j in range(NJ):
                        nc.vector.tensor_scalar(
                            sq[:, j, :],
                            xt[:, j, :],
                            2.0,
                            0.0,
                            ALU.pow,
                            ALU.add,
                            accum_out=ssr[:, j : j + 1],
                        )
                else:
                    for j in range(NJ):
                        nc.vector.scalar_tensor_tensor(
                            sq[:, j, :],
                            xt[:, j, :],
                            1.0,
                            xt[:, j, :],
                            ALU.mult,
                            ALU.mult,
                            accum_out=ssr[:, j : j + 1],
                        )

                # ---- inv = 1 / sqrt(ss + 1e-16)   (1e-16 keeps zero rows finite,
                #      matches reference x / max(||x||, 1e-8) exactly for zeros)
                snorm = npool.tile([128, NJ], F32, tag="snorm")
                nc.scalar.activation(snorm[:], ssr[:], ACTF.Sqrt, bias=eps_bias[:])
                sinv = npool.tile([128, NJ], F32, tag="sinv")
                nc.vector.reciprocal(sinv[:], snorm[:])

                # ---- xn = x * inv  (f32 -> bf16), per chunk (per-partition scalar)
                xnt = xnpool.tile([128, NJ, C], BF16, tag="xnt")
                for j in range(NJ):
                    nc.vector.tensor_scalar(
                        xnt[:, j, :],
                        xt[:, j, :],
                        sinv[:, j : j + 1],
                        None,
                        ALU.mult,
                    )
                xn_rows[h] = xnt

                if h < 2:
                    continue

                # ---- output row r (padded coords); local output index r-1
                r = h - 1
                ro = r - 1  # 0..n_out_rows-1
                xa, xb_, xc = xn_rows[r - 1], xn_rows[r], xn_rows[r + 1]
                xn_rows[r - 1] = None

                vtmp = vpool.tile([128, NJ, C], BF16, tag="vtmp")
                nc.vector.tensor_add(vtmp[:], xa[:], xc[:])
                vt = vpool.tile([128, NJ, C], BF16, tag="vt")
                nc.vector.tensor_add(vt[:], vtmp[:], xb_[:])

                # ---- horizontal box sum on PE: S = T@V + EL@V(j-1) + ER@V(j+1)
                S = psumpool.tile([128, NJ, C], F32, tag="S")
                hj = NJ // 2  # PSUM bank boundary at j=4 (512 f32)
                nc.tensor.matmul(
                    S[:, 0:hj, :], band[:], vt[:, 0:hj, :], start=True, stop=False
                )
                nc.tensor.matmul(
                    S[:, hj:NJ, :], band[:], vt[:, hj:NJ, :], start=True, stop=False
                )
                nc.tensor.matmul(
                    S[:, 1:hj, :], el[:], vt[:, 0 : hj - 1, :], start=False, stop=False
                )
                nc.tensor.matmul(
                    S[:, hj:NJ, :], el[:], vt[:, hj - 1 : NJ - 1, :],
                    start=False, stop=False,
                )
                nc.tensor.matmul(
                    S[:, 0:hj, :], er[:], vt[:, 1 : hj + 1, :], start=False, stop=True
                )
                nc.tensor.matmul(
                    S[:, hj : NJ - 1, :], er[:], vt[:, hj + 1 : NJ, :],
                    start=False, stop=True,
                )

                # ---- evacuate S to SBUF as bf16 (ACT)
                sb = sbpool.tile([128, NJ, C], BF16, tag="sbt")
                nc.scalar.activation(sb[:], S[:], ACTF.Copy)

                # ---- sim[p, j] = sum_c xn*S - 1 (fused, init = -1)
                if ro % RBLK == 0:
                    simt = simpool.tile([128, NJ * RBLK], F32, tag="simt")
                rr = ro % RBLK
                pd = pdpool.tile([128, NJ, C], BF16, tag="pd")
                for j in range(NJ):
                    col = j * RBLK + rr
                    nc.vector.scalar_tensor_tensor(
                        pd[:, j, :],
                        xb_[:, j, :],
                        1.0,
                        sb[:, j, :],
                        ALU.mult,
                        ALU.mult,
                        accum_out=simt[:, col : col + 1],
                    )

                if ro % RBLK == RBLK - 1 or ro == n_out_rows - 1:
                    blk = ro // RBLK
                    simo = simpool.tile([128, NJ * RBLK], F32, tag="simo")
                    nc.vector.tensor_scalar(
                        simo[:], simt[:], -1.0, None, ALU.add
                    )
                    nc.sync.dma_start(out_dram[blk], simo[:])

    nc.compile()
    return nc


def shard_inputs(input_image):
    """input_image [H, W, 1, C] f32 -> per-core in_maps."""
    x = np.asarray(input_image).reshape(H, W, C).astype(np.float32, copy=False)
    xp = np.zeros((H + 2, W, C), np.float32)
    xp[1 : H + 1] = x
    band, el, er = build_consts()
    in_maps = []
    for core in range(NCORES):
        lo = core * ROWS_PER_CORE
        shard = np.ascontiguousarray(xp[lo : lo + ROWS_PER_CORE + 2])
        in_maps.append({"x": shard, "band": band, "el": el, "er": er})
    return in_maps


def unshard_output(results):
    """results[i]['out'] [nblk, 128, NJ*RBLK] -> [H, W] f32."""
    out = np.empty((H, W), np.float32)
    for core in range(NCORES):
        st = np.asarray(results[core]["out"])  # [nblk, 128, NJ*RBLK]
        nblk = st.shape[0]
        st = st.reshape(nblk, 128, NJ, RBLK)  # [blk, p, j, rr]
        sim = st.transpose(0, 3, 2, 1).reshape(nblk * RBLK, W)  # [h_local, w]
        out[core * ROWS_PER_CORE : (core + 1) * ROWS_PER_CORE] = sim[:ROWS_PER_CORE]
    return out


_NC_CACHE = {}


def get_nc():
    if "nc" not in _NC_CACHE:
        _NC_CACHE["nc"] = build_bass()
    return _NC_CACHE["nc"]


def kernel(input_image):
    nc = get_nc()
    in_maps = shard_inputs(input_image)
    res = run_bass_kernel_spmd(nc, in_maps, list(range(NCORES)))
    return unshard_output(res.results)


if __name__ == "__main__":
    rng = np.random.default_rng(0)
    x = rng.standard_normal((H, W, 1, C), dtype=np.float32)
    out = kernel(x)
    print(out.shape, out.dtype, out[:2, :4])



# revision 2
# speedup vs baseline: 1.6448x; 1.6448x over previous
"""3x3 neighborhood cosine-similarity sum (minus self) on 8 TRN2 NeuronCores.

Input:  input_image [1024, 1024, 1, C=128] float32  (H, W, 1, C)
Output: sim [1024, 1024] float32

sim = <xn, BoxSum3x3(xn)> - 1, xn = x / max(||x||, eps) per pixel.

Sharding: H rows split 128/core across 8 cores; each core receives 144 rows
(its 128 + 8-row aligned halo padding, zeros outside the image).

Per-core layout: w = 8p + j  ->  SBUF tiles [128 p, R=8 rows, 8 j, 128 c].
Each partition line is 4KB-contiguous in HBM (fast DMA); horizontal w+-1 is
a free-dim shift except at j=0/7 (handled by sub/super-diagonal matmuls).

Engine split per row batch (R=8 rows):
 - cast-DMA (SWDGE)  : f32 HBM -> bf16 SBUF
 - ACT               : sq = Square(xb); sqrt(ss+eps); S~ PSUM->SBUF evac
 - DVE               : ss = tensor_reduce(sq); inv = 1/sqrt; dup inv pairs;
                       xn = inv-broadcast * xb (4D pair-broadcast TT at 2x);
                       A = xn_{r-1}+xn_{r+1}; prod = xn*S~;
                       dot = tensor_reduce(prod); sim = dot - 1
 - PE                : S~ = sum_{dh in -1,0,1} shift_dh(A_r) + shift_dh(xn_r)
                       via identity matmuls w/ shifted rhs APs accumulated in
                       PSUM + sub/super-diagonal boundary matmuls
"""

import numpy as np
import ml_dtypes

import sys

for _p in ("/opt/trn_rl_repo",):
    if _p not in sys.path:
        sys.path.insert(0, _p)

import concourse.bass as bass
import concourse.bacc as bacc
import concourse.mybir as mybir
import concourse.tile as tile
from concourse.bass_utils import run_bass_kernel_spmd

F32 = mybir.dt.float32
BF16 = mybir.dt.bfloat16
ALU = mybir.AluOpType
ACTF = mybir.ActivationFunctionType
AXIS = mybir.AxisListType

H, W, C = 1024, 1024, 128
NCORES = 8
RPC = H // NCORES          # 128 output rows per core
R = 8                      # rows per batch
NJ = 8                     # j per partition; w = 8p + j
NB_OUT = RPC // R          # 16 output batches
NB_IN = NB_OUT + 2         # 18 input batches = 144 rows (8-row halo pad each side)
G = R * NJ                 # 64 (row, j) groups per batch


def build_consts():
    ident = np.eye(128, dtype=np.float32)
    subd = np.zeros((128, 128), np.float32)
    supd = np.zeros((128, 128), np.float32)
    for p in range(127):
        subd[p, p + 1] = 1.0   # out[m] += rhs[m-1]
        supd[p + 1, p] = 1.0   # out[m] += rhs[m+1]
    bf = lambda a: a.astype(ml_dtypes.bfloat16)
    return bf(ident), bf(subd), bf(supd)


def build_bass():
    nc = bacc.Bacc(None, target_bir_lowering=False)
    x_dram = nc.declare_dram_parameter("x", [NB_IN * R, W, C], F32, isOutput=False)
    id_dram = nc.declare_dram_parameter("ident", [128, 128], BF16, isOutput=False)
    sub_dram = nc.declare_dram_parameter("subd", [128, 128], BF16, isOutput=False)
    sup_dram = nc.declare_dram_parameter("supd", [128, 128], BF16, isOutput=False)
    out_dram = nc.declare_dram_parameter("out", [NB_OUT, 128, G], F32, isOutput=True)

    with tile.TileContext(nc) as tc:
        with (
            tc.tile_pool(name="consts", bufs=1) as cpool,
            tc.tile_pool(name="xb", bufs=2) as xpool,
            tc.tile_pool(name="sq", bufs=2) as sqpool,
            tc.tile_pool(name="xn", bufs=3) as xnpool,
            tc.tile_pool(name="aa", bufs=2) as apool,
            tc.tile_pool(name="ssb", bufs=2) as ssbpool,
            tc.tile_pool(name="prod", bufs=1) as prodpool,
            tc.tile_pool(name="stat", bufs=2) as statpool,
            tc.tile_pool(name="psum", bufs=2, space="PSUM") as psumpool,
        ):
            ident = cpool.tile([128, 128], BF16, tag="ident")
            subd = cpool.tile([128, 128], BF16, tag="subd")
            supd = cpool.tile([128, 128], BF16, tag="supd")
            nc.sync.dma_start(ident[:], id_dram[:])
            nc.sync.dma_start(subd[:], sub_dram[:])
            nc.sync.dma_start(supd[:], sup_dram[:])
            eps = cpool.tile([128, 1], F32, tag="eps")
            nc.gpsimd.memset(eps[:], 1e-16)

            xn_t = [None] * NB_IN

            def emit_output_batch(ob):
                xp, xc, xx = xn_t[ob], xn_t[ob + 1], xn_t[ob + 2]
                xn_t[ob] = None

                # A = xn_{r-1} + xn_{r+1} (vertical neighbors)
                A = apool.tile([128, R, NJ, C], BF16, tag="A")
                nc.vector.tensor_add(A[:, 0], xp[:, R - 1], xc[:, 1])
                nc.vector.tensor_add(A[:, 1 : R - 1], xc[:, 0 : R - 2], xc[:, 2:R])
                nc.vector.tensor_add(A[:, R - 1], xc[:, R - 2], xx[:, 0])

                ssb = ssbpool.tile([128, R, NJ, C], BF16, tag="ssb")
                for i in range(R):
                    S = psumpool.tile([128, NJ, C], F32, tag="S")
                    Ar = A[:, i]
                    Xr = xc[:, i]
                    # A: dh=0 (starts both banks), dh=-1, dh=+1, boundaries
                    nc.tensor.matmul(S[:, 0:4], ident[:], Ar[:, 0:4], start=True, stop=False)
                    nc.tensor.matmul(S[:, 4:8], ident[:], Ar[:, 4:8], start=True, stop=False)
                    nc.tensor.matmul(S[:, 1:4], ident[:], Ar[:, 0:3], start=False, stop=False)
                    nc.tensor.matmul(S[:, 4:8], ident[:], Ar[:, 3:7], start=False, stop=False)
                    nc.tensor.matmul(S[:, 0:4], ident[:], Ar[:, 1:5], start=False, stop=False)
                    nc.tensor.matmul(S[:, 4:7], ident[:], Ar[:, 5:8], start=False, stop=False)
                    # xn_r: dh=-1, dh=+1
                    nc.tensor.matmul(S[:, 1:4], ident[:], Xr[:, 0:3], start=False, stop=False)
                    nc.tensor.matmul(S[:, 4:8], ident[:], Xr[:, 3:7], start=False, stop=False)
                    nc.tensor.matmul(S[:, 0:4], ident[:], Xr[:, 1:5], start=False, stop=False)
                    nc.tensor.matmul(S[:, 4:7], ident[:], Xr[:, 5:8], start=False, stop=False)
                    # boundary j=0 (w-1) and j=7 (w+1) for both A and xn
                    nc.tensor.matmul(S[:, 0:1], subd[:], Ar[:, 7:8], start=False, stop=False)
                    nc.tensor.matmul(S[:, 0:1], subd[:], Xr[:, 7:8], start=False, stop=False)
                    nc.tensor.matmul(S[:, 7:8], supd[:], Ar[:, 0:1], start=False, stop=False)
                    nc.tensor.matmul(S[:, 7:8], supd[:], Xr[:, 0:1], start=False, stop=False)
                    # xn_r dh=0 last, full width, carries stop
                    nc.tensor.matmul(S[:, 0:4], ident[:], Xr[:, 0:4], start=False, stop=True)
                    nc.tensor.matmul(S[:, 4:8], ident[:], Xr[:, 4:8], start=False, stop=True)

                    nc.scalar.activation(ssb[:, i], S[:], ACTF.Copy)

                prod = prodpool.tile([128, R, NJ, C], BF16, tag="prod")
                nc.vector.tensor_mul(prod[:], xc[:], ssb[:])
                dotr = statpool.tile([128, G], F32, tag="dotr")
                nc.vector.tensor_reduce(
                    dotr[:], prod[:].rearrange("p r j c -> p (r j) c"), AXIS.X, ALU.add
                )
                sim = statpool.tile([128, G], F32, tag="sim")
                nc.vector.tensor_scalar(sim[:], dotr[:], -1.0, None, ALU.add)
                nc.sync.dma_start(out_dram[ob], sim[:])

            for b in range(NB_IN):
                xb = xpool.tile([128, R, NJ, C], BF16, tag="xb")
                nc.gpsimd.dma_start(
                    xb[:],
                    x_dram[b * R : (b + 1) * R].rearrange("r (p j) c -> p r j c", p=128),
                )
                sq = sqpool.tile([128, R, NJ, C], BF16, tag="sq")
                nc.scalar.activation(sq[:], xb[:], ACTF.Square)
                ssr = statpool.tile([128, G], F32, tag="ssr")
                nc.vector.tensor_reduce(
                    ssr[:], sq[:].rearrange("p r j c -> p (r j) c"), AXIS.X, ALU.add
                )
                snorm = statpool.tile([128, G], F32, tag="snorm")
                nc.scalar.activation(snorm[:], ssr[:], ACTF.Sqrt, bias=eps[:])
                sinv = statpool.tile([128, G], F32, tag="sinv")
                nc.vector.reciprocal(sinv[:], snorm[:])
                invd = statpool.tile([128, G, 2], BF16, tag="invd")
                nc.vector.tensor_scalar(invd[:, :, 0:1], sinv[:].unsqueeze(2), 1.0, None, ALU.mult)
                nc.vector.tensor_scalar(invd[:, :, 1:2], sinv[:].unsqueeze(2), 1.0, None, ALU.mult)

                xnb = xnpool.tile([128, R, NJ, C], BF16, tag="xn")
                nc.vector.tensor_tensor(
                    xnb[:].rearrange("p r j (h two) -> p (r j) h two", two=2),
                    invd[:].unsqueeze(2).broadcast_to([128, G, C // 2, 2]),
                    xb[:].rearrange("p r j (h two) -> p (r j) h two", two=2),
                    ALU.mult,
                )
                xn_t[b] = xnb

                if b >= 2:
                    emit_output_batch(b - 2)

    nc.compile()
    return nc


def shard_inputs(input_image):
    """input_image [H, W, 1, C] f32 -> per-core in_maps (144 padded rows each)."""
    x = np.asarray(input_image).reshape(H, W, C).astype(np.float32, copy=False)
    ident, subd, supd = build_consts()
    in_maps = []
    for core in range(NCORES):
        lo = core * RPC
        shard = np.zeros((NB_IN * R, W, C), np.float32)
        # shard row i = global row (lo - 8 + i); valid range [lo-1, lo+128]
        gs = max(lo - 8, 0)
        ge = min(lo + RPC + 8, H)
        shard[gs - (lo - 8) : ge - (lo - 8)] = x[gs:ge]
        in_maps.append({"x": shard, "ident": ident, "subd": subd, "supd": supd})
    return in_maps


def unshard_output(results):
    """results[i]['out'] [NB_OUT, 128, G] -> [H, W] f32."""
    out = np.empty((H, W), np.float32)
    for core in range(NCORES):
        st = np.asarray(results[core]["out"]).reshape(NB_OUT, 128, R, NJ)
        sim = st.transpose(0, 2, 1, 3).reshape(RPC, W)  # w = 8p + j
        out[core * RPC : (core + 1) * RPC] = sim
    return out


_NC_CACHE = {}


def get_nc():
    if "nc" not in _NC_CACHE:
        _NC_CACHE["nc"] = build_bass()
    return _NC_CACHE["nc"]


def kernel(input_image):
    nc = get_nc()
    in_maps = shard_inputs(input_image)
    res = run_bass_kernel_spmd(nc, in_maps, list(range(NCORES)))
    return unshard_output(res.results)


if __name__ == "__main__":
    rng = np.random.default_rng(0)
    x = rng.standard_normal((H, W, 1, C), dtype=np.float32)
    out = kernel(x)
    print(out.shape, out.dtype, out[:2, :4])


# revision 5
# speedup vs baseline: 1.8094x; 1.1001x over previous
"""3x3 neighborhood cosine-similarity sum (minus self) on 8 TRN2 NeuronCores.

Input:  input_image [1024, 1024, 1, C=128] float32  (H, W, 1, C)
Output: sim [1024, 1024] float32

sim = <xn, BoxSum3x3(xn)> - 1, xn = x / max(||x||, eps) per pixel.

Sharding: H rows split 128/core across 8 cores; each core receives 144 rows
(its 128 + 8-row aligned halo padding, zeros outside the image).

Per-core layout: w = 8p + j  ->  SBUF tiles [128 p, R=8 rows, 8 j, 128 c].
Each partition line is 4KB-contiguous in HBM (fast DMA); horizontal w+-1 is
a free-dim shift except at j=0/7 (handled by sub/super-diagonal matmuls).

Engine split per row batch (R=8 rows):
 - cast-DMA (SWDGE)  : f32 HBM -> bf16 SBUF
 - ACT               : sq = Square(xb); sqrt(ss+eps); S~ PSUM->SBUF evac
 - DVE               : ss = tensor_reduce(sq); inv = 1/sqrt; dup inv pairs;
                       xn = inv-broadcast * xb (4D pair-broadcast TT at 2x);
                       A = xn_{r-1}+xn_{r+1}; prod = xn*S~;
                       dot = tensor_reduce(prod); sim = dot - 1
 - PE                : S~ = sum_{dh in -1,0,1} shift_dh(A_r) + shift_dh(xn_r)
                       via identity matmuls w/ shifted rhs APs accumulated in
                       PSUM + sub/super-diagonal boundary matmuls
"""

import numpy as np
import ml_dtypes

import sys

for _p in ("/opt/trn_rl_repo",):
    if _p not in sys.path:
        sys.path.insert(0, _p)

import concourse.bass as bass
import concourse.bacc as bacc
import concourse.mybir as mybir
import concourse.tile as tile
from concourse.bass_utils import run_bass_kernel_spmd

F32 = mybir.dt.float32
BF16 = mybir.dt.bfloat16
F16 = mybir.dt.bfloat16  # TEMP bisect
ALU = mybir.AluOpType
ACTF = mybir.ActivationFunctionType
AXIS = mybir.AxisListType

H, W, C = 1024, 1024, 128
NCORES = 8
RPC = H // NCORES          # 128 output rows per core
R = 8                      # rows per batch
NJ = 8                     # j per partition; w = 8p + j
NB_OUT = RPC // R          # 16 output batches
NB_IN = NB_OUT + 2         # 18 input batches = 144 rows (8-row halo pad each side)
G = R * NJ                 # 64 (row, j) groups per batch


def build_consts():
    ident = np.eye(128, dtype=np.float32)
    subd = np.zeros((128, 128), np.float32)
    supd = np.zeros((128, 128), np.float32)
    for p in range(127):
        subd[p, p + 1] = 1.0   # out[m] += rhs[m-1]
        supd[p + 1, p] = 1.0   # out[m] += rhs[m+1]
    bf = lambda a: a.astype(ml_dtypes.bfloat16)
    return bf(ident), bf(subd), bf(supd)


def build_bass():
    nc = bacc.Bacc(None, target_bir_lowering=False)
    x_dram = nc.declare_dram_parameter("x", [NB_IN * R, W, C], F32, isOutput=False)
    id_dram = nc.declare_dram_parameter("ident", [128, 128], F16, isOutput=False)
    sub_dram = nc.declare_dram_parameter("subd", [128, 128], F16, isOutput=False)
    sup_dram = nc.declare_dram_parameter("supd", [128, 128], F16, isOutput=False)
    out_dram = nc.declare_dram_parameter("out", [NB_OUT, 128, G], F32, isOutput=True)

    with tile.TileContext(nc) as tc:
        with (
            tc.tile_pool(name="consts", bufs=1) as cpool,
            tc.tile_pool(name="xb", bufs=2) as xpool,
            tc.tile_pool(name="sq", bufs=2) as sqpool,
            tc.tile_pool(name="xn", bufs=3) as xnpool,
            tc.tile_pool(name="aa", bufs=2) as apool,
            tc.tile_pool(name="ssb", bufs=2) as ssbpool,
            tc.tile_pool(name="prod", bufs=1) as prodpool,
            tc.tile_pool(name="stat", bufs=2) as statpool,
            tc.tile_pool(name="psum", bufs=2, space="PSUM") as psumpool,
        ):
            ident = cpool.tile([128, 128], F16, tag="ident")
            subd = cpool.tile([128, 128], F16, tag="subd")
            supd = cpool.tile([128, 128], F16, tag="supd")
            nc.sync.dma_start(ident[:], id_dram[:])
            nc.sync.dma_start(subd[:], sub_dram[:])
            nc.sync.dma_start(supd[:], sup_dram[:])
            eps = cpool.tile([128, 1], F32, tag="eps")
            nc.gpsimd.memset(eps[:], 1e-4)

            xn_t = [None] * NB_IN

            def fold_reduce(out_f32, t4d):
                """Sum over c (last axis, 128 wide) of t4d [128, R, NJ, C] fp16
                into out_f32 [128, G] via in-place 2x fold cascade."""
                v = t4d[:].rearrange("p r j c -> p (r j) c")
                w = C
                while w > 8:
                    h = w // 2
                    nc.vector.tensor_add(v[:, :, 0:h], v[:, :, 0:h], v[:, :, h:w])
                    w = h
                nc.vector.tensor_reduce(out_f32, v[:, :, 0:w], AXIS.X, ALU.add)


            def emit_output_batch(ob):
                xp, xc, xx = xn_t[ob], xn_t[ob + 1], xn_t[ob + 2]
                xn_t[ob] = None

                # A = xn_{r-1} + xn_{r+1} (vertical neighbors)
                A = apool.tile([128, R, NJ, C], F16, tag="A")
                nc.vector.tensor_add(A[:, 0], xp[:, R - 1], xc[:, 1])
                nc.vector.tensor_add(A[:, 1 : R - 1], xc[:, 0 : R - 2], xc[:, 2:R])
                nc.vector.tensor_add(A[:, R - 1], xc[:, R - 2], xx[:, 0])

                ssb = ssbpool.tile([128, R, NJ, C], F16, tag="ssb")
                for i in range(R):
                    S = psumpool.tile([128, NJ, C], F32, tag="S")
                    Ar = A[:, i]
                    Xr = xc[:, i]
                    # A: dh=0 (starts both banks), dh=-1, dh=+1, boundaries
                    nc.tensor.matmul(S[:, 0:4], ident[:], Ar[:, 0:4], start=True, stop=False)
                    nc.tensor.matmul(S[:, 4:8], ident[:], Ar[:, 4:8], start=True, stop=False)
                    nc.tensor.matmul(S[:, 1:4], ident[:], Ar[:, 0:3], start=False, stop=False)
                    nc.tensor.matmul(S[:, 4:8], ident[:], Ar[:, 3:7], start=False, stop=False)
                    nc.tensor.matmul(S[:, 0:4], ident[:], Ar[:, 1:5], start=False, stop=False)
                    nc.tensor.matmul(S[:, 4:7], ident[:], Ar[:, 5:8], start=False, stop=False)
                    # xn_r: dh=-1, dh=+1
                    nc.tensor.matmul(S[:, 1:4], ident[:], Xr[:, 0:3], start=False, stop=False)
                    nc.tensor.matmul(S[:, 4:8], ident[:], Xr[:, 3:7], start=False, stop=False)
                    nc.tensor.matmul(S[:, 0:4], ident[:], Xr[:, 1:5], start=False, stop=False)
                    nc.tensor.matmul(S[:, 4:7], ident[:], Xr[:, 5:8], start=False, stop=False)
                    # boundary j=0 (w-1) and j=7 (w+1) for both A and xn
                    nc.tensor.matmul(S[:, 0:1], subd[:], Ar[:, 7:8], start=False, stop=False)
                    nc.tensor.matmul(S[:, 0:1], subd[:], Xr[:, 7:8], start=False, stop=False)
                    nc.tensor.matmul(S[:, 7:8], supd[:], Ar[:, 0:1], start=False, stop=False)
                    nc.tensor.matmul(S[:, 7:8], supd[:], Xr[:, 0:1], start=False, stop=False)
                    # xn_r dh=0 last, full width, carries stop
                    nc.tensor.matmul(S[:, 0:4], ident[:], Xr[:, 0:4], start=False, stop=True)
                    nc.tensor.matmul(S[:, 4:8], ident[:], Xr[:, 4:8], start=False, stop=True)

                    nc.scalar.activation(ssb[:, i], S[:], ACTF.Copy)

                prod = prodpool.tile([128, R, NJ, C], F16, tag="prod")
                nc.vector.tensor_mul(prod[:], xc[:], ssb[:])
                dotr = statpool.tile([128, G], F32, tag="dotr")
                fold_reduce(dotr[:], prod)
                sim = statpool.tile([128, G], F32, tag="sim")
                nc.vector.tensor_scalar(sim[:], dotr[:], -1.0, None, ALU.add)
                nc.sync.dma_start(out_dram[ob], sim[:])

            for b in range(NB_IN):
                xb = xpool.tile([128, R, NJ, C], F16, tag="xb")
                nc.gpsimd.dma_start(
                    xb[:],
                    x_dram[b * R : (b + 1) * R].rearrange("r (p j) c -> p r j c", p=128),
                )
                sq = sqpool.tile([128, R, NJ, C], F16, tag="sq")
                nc.scalar.activation(sq[:], xb[:], ACTF.Square)
                ssr = statpool.tile([128, G], F32, tag="ssr")
                fold_reduce(ssr[:], sq)
                snorm = statpool.tile([128, G], F32, tag="snorm")
                nc.scalar.activation(snorm[:], ssr[:], ACTF.Sqrt, bias=eps[:])
                sinv = statpool.tile([128, G], F32, tag="sinv")
                nc.vector.reciprocal(sinv[:], snorm[:])
                invd = statpool.tile([128, G, 2], F16, tag="invd")
                nc.vector.tensor_scalar(invd[:, :, 0:1], sinv[:].unsqueeze(2), 1.0, None, ALU.mult)
                nc.vector.tensor_scalar(invd[:, :, 1:2], sinv[:].unsqueeze(2), 1.0, None, ALU.mult)

                xnb = xnpool.tile([128, R, NJ, C], F16, tag="xn")
                nc.vector.tensor_tensor(
                    xnb[:].rearrange("p r j (h two) -> p (r j) h two", two=2),
                    invd[:].unsqueeze(2).broadcast_to([128, G, C // 2, 2]),
                    xb[:].rearrange("p r j (h two) -> p (r j) h two", two=2),
                    ALU.mult,
                )
                xn_t[b] = xnb

                if b >= 2:
                    emit_output_batch(b - 2)

    nc.compile()
    return nc


def shard_inputs(input_image):
    """input_image [H, W, 1, C] f32 -> per-core in_maps (144 padded rows each)."""
    x = np.asarray(input_image).reshape(H, W, C).astype(np.float32, copy=False)
    ident, subd, supd = build_consts()
    in_maps = []
    for core in range(NCORES):
        lo = core * RPC
        shard = np.zeros((NB_IN * R, W, C), np.float32)
        # shard row i = global row (lo - 8 + i); valid range [lo-1, lo+128]
        gs = max(lo - 8, 0)
        ge = min(lo + RPC + 8, H)
        shard[gs - (lo - 8) : ge - (lo - 8)] = x[gs:ge]
        in_maps.append({"x": shard, "ident": ident, "subd": subd, "supd": supd})
    return in_maps


def unshard_output(results):
    """results[i]['out'] [NB_OUT, 128, G] -> [H, W] f32."""
    out = np.empty((H, W), np.float32)
    for core in range(NCORES):
        st = np.asarray(results[core]["out"]).reshape(NB_OUT, 128, R, NJ)
        sim = st.transpose(0, 2, 1, 3).reshape(RPC, W)  # w = 8p + j
        out[core * RPC : (core + 1) * RPC] = sim
    return out


_NC_CACHE = {}


def get_nc():
    if "nc" not in _NC_CACHE:
        _NC_CACHE["nc"] = build_bass()
    return _NC_CACHE["nc"]


def kernel(input_image):
    nc = get_nc()
    in_maps = shard_inputs(input_image)
    res = run_bass_kernel_spmd(nc, in_maps, list(range(NCORES)))
    return unshard_output(res.results)


if __name__ == "__main__":
    rng = np.random.default_rng(0)
    x = rng.standard_normal((H, W, 1, C), dtype=np.float32)
    out = kernel(x)
    print(out.shape, out.dtype, out[:2, :4])


# revision 6
# speedup vs baseline: 1.8978x; 1.0488x over previous
"""3x3 neighborhood cosine-similarity sum (minus self) on 8 TRN2 NeuronCores.

Input:  input_image [1024, 1024, 1, C=128] float32  (H, W, 1, C)
Output: sim [1024, 1024] float32

sim = <xn, BoxSum3x3(xn)> - 1, xn = x / max(||x||, eps) per pixel.

Sharding: H rows split 128/core across 8 cores; each core receives 144 rows
(its 128 + 8-row aligned halo padding, zeros outside the image).

Per-core layout: w = 8p + j  ->  SBUF tiles [128 p, R=8 rows, 8 j, 128 c].
Each partition line is 4KB-contiguous in HBM (fast DMA); horizontal w+-1 is
a free-dim shift except at j=0/7 (handled by sub/super-diagonal matmuls).

Engine split per row batch (R=8 rows):
 - cast-DMA (SWDGE)  : f32 HBM -> bf16 SBUF
 - ACT               : sq = Square(xb); sqrt(ss+eps); S~ PSUM->SBUF evac
 - DVE               : ss = tensor_reduce(sq); inv = 1/sqrt; dup inv pairs;
                       xn = inv-broadcast * xb (4D pair-broadcast TT at 2x);
                       A = xn_{r-1}+xn_{r+1}; prod = xn*S~;
                       dot = tensor_reduce(prod); sim = dot - 1
 - PE                : S~ = sum_{dh in -1,0,1} shift_dh(A_r) + shift_dh(xn_r)
                       via identity matmuls w/ shifted rhs APs accumulated in
                       PSUM + sub/super-diagonal boundary matmuls
"""

import numpy as np
import ml_dtypes

import sys

for _p in ("/opt/trn_rl_repo",):
    if _p not in sys.path:
        sys.path.insert(0, _p)

import concourse.bass as bass
import concourse.bacc as bacc
import concourse.mybir as mybir
import concourse.tile as tile
from concourse.bass_utils import run_bass_kernel_spmd

F32 = mybir.dt.float32
BF16 = mybir.dt.bfloat16
import os
F16 = mybir.dt.float16 if os.environ.get('K_FP16','1')=='1' else mybir.dt.bfloat16
ALU = mybir.AluOpType
ACTF = mybir.ActivationFunctionType
AXIS = mybir.AxisListType

H, W, C = 1024, 1024, 128
NCORES = 8
RPC = H // NCORES          # 128 output rows per core
R = 8                      # rows per batch
NJ = 8                     # j per partition; w = 8p + j
NB_OUT = RPC // R          # 16 output batches
NB_IN = NB_OUT + 2         # 18 input batches = 144 rows (8-row halo pad each side)
G = R * NJ                 # 64 (row, j) groups per batch


def build_consts():
    ident = np.eye(128, dtype=np.float32)
    subd = np.zeros((128, 128), np.float32)
    supd = np.zeros((128, 128), np.float32)
    for p in range(127):
        subd[p, p + 1] = 1.0   # out[m] += rhs[m-1]
        supd[p + 1, p] = 1.0   # out[m] += rhs[m+1]
    import os as _os
    bf = lambda a: a.astype(np.float16 if _os.environ.get('K_FP16','1')=='1' else ml_dtypes.bfloat16)
    return bf(ident), bf(subd), bf(supd)


def build_bass():
    nc = bacc.Bacc(None, target_bir_lowering=False)
    x_dram = nc.declare_dram_parameter("x", [NB_IN * R, W, C], F32, isOutput=False)
    id_dram = nc.declare_dram_parameter("ident", [128, 128], F16, isOutput=False)
    sub_dram = nc.declare_dram_parameter("subd", [128, 128], F16, isOutput=False)
    sup_dram = nc.declare_dram_parameter("supd", [128, 128], F16, isOutput=False)
    out_dram = nc.declare_dram_parameter("out", [NB_OUT, 128, G], F32, isOutput=True)

    with tile.TileContext(nc) as tc:
        with (
            tc.tile_pool(name="consts", bufs=1) as cpool,
            tc.tile_pool(name="xb", bufs=2) as xpool,
            tc.tile_pool(name="sq", bufs=2) as sqpool,
            tc.tile_pool(name="xn", bufs=3) as xnpool,
            tc.tile_pool(name="aa", bufs=2) as apool,
            tc.tile_pool(name="ssb", bufs=2) as ssbpool,
            tc.tile_pool(name="prod", bufs=1) as prodpool,
            tc.tile_pool(name="stat", bufs=2) as statpool,
            tc.tile_pool(name="psum", bufs=2, space="PSUM") as psumpool,
        ):
            ident = cpool.tile([128, 128], F16, tag="ident")
            subd = cpool.tile([128, 128], F16, tag="subd")
            supd = cpool.tile([128, 128], F16, tag="supd")
            nc.sync.dma_start(ident[:], id_dram[:])
            nc.sync.dma_start(subd[:], sub_dram[:])
            nc.sync.dma_start(supd[:], sup_dram[:])
            eps = cpool.tile([128, 1], F32, tag="eps")
            nc.gpsimd.memset(eps[:], 1e-4)

            xn_t = [None] * NB_IN

            def fold_reduce(out_f32, t4d):
                """Sum over c (last axis, 128 wide) of t4d [128, R, NJ, C] fp16
                into out_f32 [128, G] via in-place 2x fold cascade."""
                v = t4d[:].rearrange("p r j c -> p (r j) c")
                w = C
                while w > 8:
                    h = w // 2
                    nc.vector.tensor_add(v[:, :, 0:h], v[:, :, 0:h], v[:, :, h:w])
                    w = h
                nc.vector.tensor_reduce(out_f32, v[:, :, 0:w], AXIS.X, ALU.add)


            def emit_output_batch(ob):
                xp, xc, xx = xn_t[ob], xn_t[ob + 1], xn_t[ob + 2]
                xn_t[ob] = None

                # A = xn_{r-1} + xn_{r+1} (vertical neighbors)
                A = apool.tile([128, R, NJ, C], F16, tag="A")
                nc.vector.tensor_add(A[:, 0], xp[:, R - 1], xc[:, 1])
                nc.vector.tensor_add(A[:, 1 : R - 1], xc[:, 0 : R - 2], xc[:, 2:R])
                nc.vector.tensor_add(A[:, R - 1], xc[:, R - 2], xx[:, 0])

                ssb = ssbpool.tile([128, R, NJ, C], F16, tag="ssb")
                for i in range(R):
                    S = psumpool.tile([128, NJ, C], F32, tag="S")
                    Ar = A[:, i]
                    Xr = xc[:, i]
                    # A: dh=0 (starts both banks), dh=-1, dh=+1, boundaries
                    nc.tensor.matmul(S[:, 0:4], ident[:], Ar[:, 0:4], start=True, stop=False)
                    nc.tensor.matmul(S[:, 4:8], ident[:], Ar[:, 4:8], start=True, stop=False)
                    nc.tensor.matmul(S[:, 1:4], ident[:], Ar[:, 0:3], start=False, stop=False)
                    nc.tensor.matmul(S[:, 4:8], ident[:], Ar[:, 3:7], start=False, stop=False)
                    nc.tensor.matmul(S[:, 0:4], ident[:], Ar[:, 1:5], start=False, stop=False)
                    nc.tensor.matmul(S[:, 4:7], ident[:], Ar[:, 5:8], start=False, stop=False)
                    # xn_r: dh=-1, dh=+1
                    nc.tensor.matmul(S[:, 1:4], ident[:], Xr[:, 0:3], start=False, stop=False)
                    nc.tensor.matmul(S[:, 4:8], ident[:], Xr[:, 3:7], start=False, stop=False)
                    nc.tensor.matmul(S[:, 0:4], ident[:], Xr[:, 1:5], start=False, stop=False)
                    nc.tensor.matmul(S[:, 4:7], ident[:], Xr[:, 5:8], start=False, stop=False)
                    # boundary j=0 (w-1) and j=7 (w+1) for both A and xn
                    nc.tensor.matmul(S[:, 0:1], subd[:], Ar[:, 7:8], start=False, stop=False)
                    nc.tensor.matmul(S[:, 0:1], subd[:], Xr[:, 7:8], start=False, stop=False)
                    nc.tensor.matmul(S[:, 7:8], supd[:], Ar[:, 0:1], start=False, stop=False)
                    nc.tensor.matmul(S[:, 7:8], supd[:], Xr[:, 0:1], start=False, stop=False)
                    # xn_r dh=0 last, full width, carries stop
                    nc.tensor.matmul(S[:, 0:4], ident[:], Xr[:, 0:4], start=False, stop=True)
                    nc.tensor.matmul(S[:, 4:8], ident[:], Xr[:, 4:8], start=False, stop=True)

                    nc.scalar.activation(ssb[:, i], S[:], ACTF.Copy)

                prod = prodpool.tile([128, R, NJ, C], F16, tag="prod")
                nc.vector.tensor_mul(prod[:], xc[:], ssb[:])
                dotr = statpool.tile([128, G], F32, tag="dotr")
                fold_reduce(dotr[:], prod)
                sim = statpool.tile([128, G], F32, tag="sim")
                nc.vector.tensor_scalar(sim[:], dotr[:], -1.0, None, ALU.add)
                nc.sync.dma_start(out_dram[ob], sim[:])

            for b in range(NB_IN):
                xb = xpool.tile([128, R, NJ, C], F16, tag="xb")
                nc.gpsimd.dma_start(
                    xb[:],
                    x_dram[b * R : (b + 1) * R].rearrange("r (p j) c -> p r j c", p=128),
                )
                sq = sqpool.tile([128, R, NJ, C], F16, tag="sq")
                nc.scalar.activation(sq[:], xb[:], ACTF.Square)
                ssr = statpool.tile([128, G], F32, tag="ssr")
                fold_reduce(ssr[:], sq)
                snorm = statpool.tile([128, G], F32, tag="snorm")
                nc.scalar.activation(snorm[:], ssr[:], ACTF.Sqrt, bias=eps[:])
                sinv = statpool.tile([128, G], F32, tag="sinv")
                nc.vector.reciprocal(sinv[:], snorm[:])
                invd = statpool.tile([128, G, 2], F16, tag="invd")
                nc.vector.tensor_scalar(invd[:, :, 0:1], sinv[:].unsqueeze(2), 1.0, None, ALU.mult)
                nc.vector.tensor_scalar(invd[:, :, 1:2], sinv[:].unsqueeze(2), 1.0, None, ALU.mult)

                xnb = xnpool.tile([128, R, NJ, C], F16, tag="xn")
                nc.vector.tensor_tensor(
                    xnb[:].rearrange("p r j (h two) -> p (r j) h two", two=2),
                    invd[:].unsqueeze(2).broadcast_to([128, G, C // 2, 2]),
                    xb[:].rearrange("p r j (h two) -> p (r j) h two", two=2),
                    ALU.mult,
                )
                xn_t[b] = xnb

                if b >= 2:
                    emit_output_batch(b - 2)

    nc.compile()
    return nc


def shard_inputs(input_image):
    """input_image [H, W, 1, C] f32 -> per-core in_maps (144 padded rows each)."""
    x = np.asarray(input_image).reshape(H, W, C).astype(np.float32, copy=False)
    ident, subd, supd = build_consts()
    in_maps = []
    for core in range(NCORES):
        lo = core * RPC
        shard = np.zeros((NB_IN * R, W, C), np.float32)
        # shard row i = global row (lo - 8 + i); valid range [lo-1, lo+128]
        gs = max(lo - 8, 0)
        ge = min(lo + RPC + 8, H)
        shard[gs - (lo - 8) : ge - (lo - 8)] = x[gs:ge]
        in_maps.append({"x": shard, "ident": ident, "subd": subd, "supd": supd})
    return in_maps


def unshard_output(results):
    """results[i]['out'] [NB_OUT, 128, G] -> [H, W] f32."""
    out = np.empty((H, W), np.float32)
    for core in range(NCORES):
        st = np.asarray(results[core]["out"]).reshape(NB_OUT, 128, R, NJ)
        sim = st.transpose(0, 2, 1, 3).reshape(RPC, W)  # w = 8p + j
        out[core * RPC : (core + 1) * RPC] = sim
    return out


_NC_CACHE = {}


def get_nc():
    if "nc" not in _NC_CACHE:
        _NC_CACHE["nc"] = build_bass()
    return _NC_CACHE["nc"]


def kernel(input_image):
    nc = get_nc()
    in_maps = shard_inputs(input_image)
    res = run_bass_kernel_spmd(nc, in_maps, list(range(NCORES)))
    return unshard_output(res.results)


if __name__ == "__main__":
    rng = np.random.default_rng(0)
    x = rng.standard_normal((H, W, 1, C), dtype=np.float32)
    out = kernel(x)
    print(out.shape, out.dtype, out[:2, :4])


# revision 7
# speedup vs baseline: 2.1631x; 1.1398x over previous
"""3x3 neighborhood cosine-similarity sum (minus self) on 8 TRN2 NeuronCores.

Input:  input_image [1024, 1024, 1, C=128] float32  (H, W, 1, C)
Output: sim [1024, 1024] float32

sim = <xn, BoxSum3x3(xn)> - 1, xn = x / max(||x||, eps) per pixel.

Sharding: H rows split 128/core across 8 cores; each core receives 144 rows
(its 128 + 8-row aligned halo padding, zeros outside the image).

Per-core layout: w = 8p + j  ->  SBUF tiles [128 p, R=8 rows, 8 j, 128 c].
Each partition line is 4KB-contiguous in HBM (fast DMA); horizontal w+-1 is
a free-dim shift except at j=0/7 (handled by sub/super-diagonal matmuls).

Engine split per row batch (R=8 rows):
 - cast-DMA (SWDGE)  : f32 HBM -> bf16 SBUF
 - ACT               : sq = Square(xb); sqrt(ss+eps); S~ PSUM->SBUF evac
 - DVE               : ss = tensor_reduce(sq); inv = 1/sqrt; dup inv pairs;
                       xn = inv-broadcast * xb (4D pair-broadcast TT at 2x);
                       A = xn_{r-1}+xn_{r+1}; prod = xn*S~;
                       dot = tensor_reduce(prod); sim = dot - 1
 - PE                : S~ = sum_{dh in -1,0,1} shift_dh(A_r) + shift_dh(xn_r)
                       via identity matmuls w/ shifted rhs APs accumulated in
                       PSUM + sub/super-diagonal boundary matmuls
"""

import numpy as np
import ml_dtypes

import sys

for _p in ("/opt/trn_rl_repo",):
    if _p not in sys.path:
        sys.path.insert(0, _p)

import concourse.bass as bass
import concourse.bacc as bacc
import concourse.mybir as mybir
import concourse.tile as tile
from concourse.bass_utils import run_bass_kernel_spmd

F32 = mybir.dt.float32
BF16 = mybir.dt.bfloat16
import os
F16 = mybir.dt.float16 if os.environ.get('K_FP16','1')=='1' else mybir.dt.bfloat16
ALU = mybir.AluOpType
ACTF = mybir.ActivationFunctionType
AXIS = mybir.AxisListType

H, W, C = 1024, 1024, 128
NCORES = 8
RPC = H // NCORES          # 128 output rows per core
R = 8                      # rows per batch
NJ = 8                     # j per partition; w = 8p + j
NB_OUT = RPC // R          # 16 output batches
NB_IN = NB_OUT + 2         # 18 input batches = 144 rows (8-row halo pad each side)
G = R * NJ                 # 64 (row, j) groups per batch


def build_consts():
    ident = np.eye(128, dtype=np.float32)
    subd = np.zeros((128, 128), np.float32)
    supd = np.zeros((128, 128), np.float32)
    for p in range(127):
        subd[p, p + 1] = 1.0   # out[m] += rhs[m-1]
        supd[p + 1, p] = 1.0   # out[m] += rhs[m+1]
    import os as _os
    bf = lambda a: a.astype(np.float16 if _os.environ.get('K_FP16','1')=='1' else ml_dtypes.bfloat16)
    return bf(ident), bf(subd), bf(supd)


def build_bass():
    nc = bacc.Bacc(None, target_bir_lowering=False)
    x_dram = nc.declare_dram_parameter("x", [NB_IN * R, W, C], F32, isOutput=False)
    id_dram = nc.declare_dram_parameter("ident", [128, 128], F16, isOutput=False)
    sub_dram = nc.declare_dram_parameter("subd", [128, 128], F16, isOutput=False)
    sup_dram = nc.declare_dram_parameter("supd", [128, 128], F16, isOutput=False)
    out_dram = nc.declare_dram_parameter("out", [NB_OUT, 128, G], F32, isOutput=True)

    with tile.TileContext(nc) as tc:
        with (
            tc.tile_pool(name="consts", bufs=1) as cpool,
            tc.tile_pool(name="xb", bufs=2) as xpool,
            tc.tile_pool(name="sq", bufs=1) as sqpool,
            tc.tile_pool(name="xn", bufs=4) as xnpool,
            tc.tile_pool(name="aa", bufs=2) as apool,
            tc.tile_pool(name="ssb", bufs=2) as ssbpool,
            tc.tile_pool(name="prod", bufs=1) as prodpool,
            tc.tile_pool(name="stat", bufs=2) as statpool,
            tc.tile_pool(name="psum", bufs=2, space="PSUM") as psumpool,
        ):
            ident = cpool.tile([128, 128], F16, tag="ident")
            subd = cpool.tile([128, 128], F16, tag="subd")
            supd = cpool.tile([128, 128], F16, tag="supd")
            nc.sync.dma_start(ident[:], id_dram[:])
            nc.sync.dma_start(subd[:], sub_dram[:])
            nc.sync.dma_start(supd[:], sup_dram[:])
            eps = cpool.tile([128, 1], F32, tag="eps")
            nc.gpsimd.memset(eps[:], 1e-4)

            xn_t = [None] * NB_IN

            def fold_reduce(out_f32, t4d):
                """Sum over c (last axis, 128 wide) of t4d [128, R, NJ, C] fp16
                into out_f32 [128, G] via in-place 2x fold cascade."""
                v = t4d[:].rearrange("p r j c -> p (r j) c")
                w = C
                while w > 8:
                    h = w // 2
                    nc.vector.tensor_add(v[:, :, 0:h], v[:, :, 0:h], v[:, :, h:w])
                    w = h
                nc.vector.tensor_reduce(out_f32, v[:, :, 0:w], AXIS.X, ALU.add)


            def emit_output_batch(ob):
                xp, xc, xx = xn_t[ob], xn_t[ob + 1], xn_t[ob + 2]
                xn_t[ob] = None

                # A = xn_{r-1} + xn_{r+1} (vertical neighbors)
                A = apool.tile([128, R, NJ, C], F16, tag="A")
                nc.vector.tensor_add(A[:, 0], xp[:, R - 1], xc[:, 1])
                nc.vector.tensor_add(A[:, 1 : R - 1], xc[:, 0 : R - 2], xc[:, 2:R])
                nc.vector.tensor_add(A[:, R - 1], xc[:, R - 2], xx[:, 0])

                ssb = ssbpool.tile([128, R, NJ, C], F16, tag="ssb")
                for i2 in range(R // 2):
                    S2 = psumpool.tile([128, 2, NJ, C], F32, tag="S")
                    for ii in range(2):
                        i = 2 * i2 + ii
                        S = S2[:, ii]
                        Ar = A[:, i]
                        Xr = xc[:, i]
                        # A: dh=0 (starts both banks), dh=-1, dh=+1, boundaries
                        nc.tensor.matmul(S[:, 0:4], ident[:], Ar[:, 0:4], start=True, stop=False)
                        nc.tensor.matmul(S[:, 4:8], ident[:], Ar[:, 4:8], start=True, stop=False)
                        nc.tensor.matmul(S[:, 1:4], ident[:], Ar[:, 0:3], start=False, stop=False)
                        nc.tensor.matmul(S[:, 4:8], ident[:], Ar[:, 3:7], start=False, stop=False)
                        nc.tensor.matmul(S[:, 0:4], ident[:], Ar[:, 1:5], start=False, stop=False)
                        nc.tensor.matmul(S[:, 4:7], ident[:], Ar[:, 5:8], start=False, stop=False)
                        # xn_r: dh=-1, dh=+1
                        nc.tensor.matmul(S[:, 1:4], ident[:], Xr[:, 0:3], start=False, stop=False)
                        nc.tensor.matmul(S[:, 4:8], ident[:], Xr[:, 3:7], start=False, stop=False)
                        nc.tensor.matmul(S[:, 0:4], ident[:], Xr[:, 1:5], start=False, stop=False)
                        nc.tensor.matmul(S[:, 4:7], ident[:], Xr[:, 5:8], start=False, stop=False)
                        # boundary j=0 (w-1) and j=7 (w+1) for both A and xn
                        nc.tensor.matmul(S[:, 0:1], subd[:], Ar[:, 7:8], start=False, stop=False)
                        nc.tensor.matmul(S[:, 0:1], subd[:], Xr[:, 7:8], start=False, stop=False)
                        nc.tensor.matmul(S[:, 7:8], supd[:], Ar[:, 0:1], start=False, stop=False)
                        nc.tensor.matmul(S[:, 7:8], supd[:], Xr[:, 0:1], start=False, stop=False)
                        # xn_r dh=0 last, full width, carries stop
                        nc.tensor.matmul(S[:, 0:4], ident[:], Xr[:, 0:4], start=False, stop=True)
                        nc.tensor.matmul(S[:, 4:8], ident[:], Xr[:, 4:8], start=False, stop=True)

                    nc.scalar.activation(ssb[:, 2 * i2 : 2 * i2 + 2], S2[:], ACTF.Copy)

                prod = prodpool.tile([128, R, NJ, C], F16, tag="prod")
                nc.vector.tensor_mul(prod[:], xc[:], ssb[:])
                dotr = statpool.tile([128, G], F32, tag="dotr")
                fold_reduce(dotr[:], prod)
                sim = statpool.tile([128, G], F32, tag="sim")
                nc.vector.tensor_scalar(sim[:], dotr[:], -1.0, None, ALU.add)
                nc.sync.dma_start(out_dram[ob], sim[:])

            for b in range(NB_IN):
                xb = xpool.tile([128, R, NJ, C], F16, tag="xb")
                nc.gpsimd.dma_start(
                    xb[:],
                    x_dram[b * R : (b + 1) * R].rearrange("r (p j) c -> p r j c", p=128),
                )
                sq = sqpool.tile([128, R, NJ, C], F16, tag="sq")
                nc.scalar.activation(sq[:], xb[:], ACTF.Square)
                ssr = statpool.tile([128, G], F32, tag="ssr")
                fold_reduce(ssr[:], sq)
                snorm = statpool.tile([128, G], F32, tag="snorm")
                nc.scalar.activation(snorm[:], ssr[:], ACTF.Sqrt, bias=eps[:])
                sinv = statpool.tile([128, G], F32, tag="sinv")
                nc.vector.reciprocal(sinv[:], snorm[:])
                invd = statpool.tile([128, G, 2], F16, tag="invd")
                nc.vector.tensor_scalar(invd[:, :, 0:1], sinv[:].unsqueeze(2), 1.0, None, ALU.mult)
                nc.vector.tensor_scalar(invd[:, :, 1:2], sinv[:].unsqueeze(2), 1.0, None, ALU.mult)

                xnb = xnpool.tile([128, R, NJ, C], F16, tag="xn")
                nc.vector.tensor_tensor(
                    xnb[:].rearrange("p r j (h two) -> p (r j) h two", two=2),
                    invd[:].unsqueeze(2).broadcast_to([128, G, C // 2, 2]),
                    xb[:].rearrange("p r j (h two) -> p (r j) h two", two=2),
                    ALU.mult,
                )
                xn_t[b] = xnb

                if b >= 2:
                    emit_output_batch(b - 2)

    nc.compile()
    return nc


def shard_inputs(input_image):
    """input_image [H, W, 1, C] f32 -> per-core in_maps (144 padded rows each)."""
    x = np.asarray(input_image).reshape(H, W, C).astype(np.float32, copy=False)
    ident, subd, supd = build_consts()
    in_maps = []
    for core in range(NCORES):
        lo = core * RPC
        shard = np.zeros((NB_IN * R, W, C), np.float32)
        # shard row i = global row (lo - 8 + i); valid range [lo-1, lo+128]
        gs = max(lo - 8, 0)
        ge = min(lo + RPC + 8, H)
        shard[gs - (lo - 8) : ge - (lo - 8)] = x[gs:ge]
        in_maps.append({"x": shard, "ident": ident, "subd": subd, "supd": supd})
    return in_maps


def unshard_output(results):
    """results[i]['out'] [NB_OUT, 128, G] -> [H, W] f32."""
    out = np.empty((H, W), np.float32)
    for core in range(NCORES):
        st = np.asarray(results[core]["out"]).reshape(NB_OUT, 128, R, NJ)
        sim = st.transpose(0, 2, 1, 3).reshape(RPC, W)  # w = 8p + j
        out[core * RPC : (core + 1) * RPC] = sim
    return out


_NC_CACHE = {}


def get_nc():
    if "nc" not in _NC_CACHE:
        _NC_CACHE["nc"] = build_bass()
    return _NC_CACHE["nc"]


def kernel(input_image):
    nc = get_nc()
    in_maps = shard_inputs(input_image)
    res = run_bass_kernel_spmd(nc, in_maps, list(range(NCORES)))
    return unshard_output(res.results)


if __name__ == "__main__":
    rng = np.random.default_rng(0)
    x = rng.standard_normal((H, W, 1, C), dtype=np.float32)
    out = kernel(x)
    print(out.shape, out.dtype, out[:2, :4])


# revision 8
# speedup vs baseline: 2.3439x; 1.0836x over previous
"""3x3 neighborhood cosine-similarity sum (minus self) on 8 TRN2 NeuronCores.

Input:  input_image [1024, 1024, 1, C=128] float32  (H, W, 1, C)
Output: sim [1024, 1024] float32

sim = <xn, BoxSum3x3(xn)> - 1, xn = x / max(||x||, eps) per pixel.

Sharding: H rows split 128/core across 8 cores; each core receives 144 rows
(its 128 + 8-row aligned halo padding, zeros outside the image).

Per-core layout: w = 8p + j  ->  SBUF tiles [128 p, R=8 rows, 8 j, 128 c].
Each partition line is 4KB-contiguous in HBM (fast DMA); horizontal w+-1 is
a free-dim shift except at j=0/7 (handled by sub/super-diagonal matmuls).

Engine split per row batch (R=8 rows):
 - cast-DMA (SWDGE)  : f32 HBM -> bf16 SBUF
 - ACT               : sq = Square(xb); sqrt(ss+eps); S~ PSUM->SBUF evac
 - DVE               : ss = tensor_reduce(sq); inv = 1/sqrt; dup inv pairs;
                       xn = inv-broadcast * xb (4D pair-broadcast TT at 2x);
                       A = xn_{r-1}+xn_{r+1}; prod = xn*S~;
                       dot = tensor_reduce(prod); sim = dot - 1
 - PE                : S~ = sum_{dh in -1,0,1} shift_dh(A_r) + shift_dh(xn_r)
                       via identity matmuls w/ shifted rhs APs accumulated in
                       PSUM + sub/super-diagonal boundary matmuls
"""

import numpy as np
import ml_dtypes

import sys

for _p in ("/opt/trn_rl_repo",):
    if _p not in sys.path:
        sys.path.insert(0, _p)

import concourse.bass as bass
import concourse.bacc as bacc
import concourse.mybir as mybir
import concourse.tile as tile
from concourse.bass_utils import run_bass_kernel_spmd

F32 = mybir.dt.float32
BF16 = mybir.dt.bfloat16
import os
F16 = mybir.dt.float16 if os.environ.get('K_FP16','1')=='1' else mybir.dt.bfloat16
ALU = mybir.AluOpType
ACTF = mybir.ActivationFunctionType
AXIS = mybir.AxisListType

H, W, C = 1024, 1024, 128
NCORES = 8
RPC = H // NCORES          # 128 output rows per core
R = 8                      # rows per batch
NJ = 8                     # j per partition; w = 8p + j
NB_OUT = RPC // R          # 16 output batches
NB_IN = NB_OUT + 2         # 18 input batches = 144 rows (8-row halo pad each side)
G = R * NJ                 # 64 (row, j) groups per batch


def build_consts():
    ident = np.eye(128, dtype=np.float32)
    subd = np.zeros((128, 128), np.float32)
    supd = np.zeros((128, 128), np.float32)
    for p in range(127):
        subd[p, p + 1] = 1.0   # out[m] += rhs[m-1]
        supd[p + 1, p] = 1.0   # out[m] += rhs[m+1]
    import os as _os
    bf = lambda a: a.astype(np.float16 if _os.environ.get('K_FP16','1')=='1' else ml_dtypes.bfloat16)
    return bf(ident), bf(subd), bf(supd)


def build_bass():
    nc = bacc.Bacc(None, target_bir_lowering=False)
    x_dram = nc.declare_dram_parameter("x", [NB_IN * R, W, C], F32, isOutput=False)
    id_dram = nc.declare_dram_parameter("ident", [128, 128], F16, isOutput=False)
    sub_dram = nc.declare_dram_parameter("subd", [128, 128], F16, isOutput=False)
    sup_dram = nc.declare_dram_parameter("supd", [128, 128], F16, isOutput=False)
    out_dram = nc.declare_dram_parameter("out", [NB_OUT, 128, G], F32, isOutput=True)

    with tile.TileContext(nc) as tc:
        with (
            tc.tile_pool(name="consts", bufs=1) as cpool,
            tc.tile_pool(name="xb", bufs=2) as xpool,
            tc.tile_pool(name="sq", bufs=1) as sqpool,
            tc.tile_pool(name="xn", bufs=4) as xnpool,
            tc.tile_pool(name="aa", bufs=2) as apool,
            tc.tile_pool(name="ssb", bufs=2) as ssbpool,
            tc.tile_pool(name="prod", bufs=1) as prodpool,
            tc.tile_pool(name="stat", bufs=2) as statpool,
            tc.tile_pool(name="psum", bufs=2, space="PSUM") as psumpool,
        ):
            ident = cpool.tile([128, 128], F16, tag="ident")
            subd = cpool.tile([128, 128], F16, tag="subd")
            supd = cpool.tile([128, 128], F16, tag="supd")
            nc.sync.dma_start(ident[:], id_dram[:])
            nc.sync.dma_start(subd[:], sub_dram[:])
            nc.sync.dma_start(supd[:], sup_dram[:])
            eps = cpool.tile([128, 1], F32, tag="eps")
            nc.gpsimd.memset(eps[:], 1e-4)

            xn_t = [None] * NB_IN

            def fold_reduce(out_f32, t4d):
                """Sum over c (last axis, 128 wide) of t4d [128, R, NJ, C] fp16
                into out_f32 [128, G] via in-place 2x fold cascade."""
                v = t4d[:].rearrange("p r j c -> p (r j) c")
                w = C
                while w > 8:
                    h = w // 2
                    nc.vector.tensor_add(v[:, :, 0:h], v[:, :, 0:h], v[:, :, h:w])
                    w = h
                nc.vector.tensor_reduce(out_f32, v[:, :, 0:w], AXIS.X, ALU.add)


            def emit_output_batch(ob):
                xp, xc, xx = xn_t[ob], xn_t[ob + 1], xn_t[ob + 2]
                xn_t[ob] = None

                # A = xn_{r-1} + xn_{r+1} (vertical neighbors)
                A = apool.tile([128, R, NJ, C], F16, tag="A")
                nc.vector.tensor_add(A[:, 0], xp[:, R - 1], xc[:, 1])
                nc.vector.tensor_add(A[:, 1 : R - 1], xc[:, 0 : R - 2], xc[:, 2:R])
                nc.vector.tensor_add(A[:, R - 1], xc[:, R - 2], xx[:, 0])

                ssb = ssbpool.tile([128, R, NJ, C], F16, tag="ssb")
                for i2 in range(R // 2):
                    S2 = psumpool.tile([128, 2, NJ, C], F32, tag="S")
                    for ii in range(2):
                        i = 2 * i2 + ii
                        S = S2[:, ii]
                        Ar = A[:, i]
                        Xr = xc[:, i]
                        # A: dh=0 (starts both banks), dh=-1, dh=+1, boundaries
                        nc.tensor.matmul(S[:, 0:4], ident[:], Ar[:, 0:4], start=True, stop=False)
                        nc.tensor.matmul(S[:, 4:8], ident[:], Ar[:, 4:8], start=True, stop=False)
                        nc.tensor.matmul(S[:, 1:4], ident[:], Ar[:, 0:3], start=False, stop=False)
                        nc.tensor.matmul(S[:, 4:8], ident[:], Ar[:, 3:7], start=False, stop=False)
                        nc.tensor.matmul(S[:, 0:4], ident[:], Ar[:, 1:5], start=False, stop=False)
                        nc.tensor.matmul(S[:, 4:7], ident[:], Ar[:, 5:8], start=False, stop=False)
                        # xn_r: dh=-1, dh=+1
                        nc.tensor.matmul(S[:, 1:4], ident[:], Xr[:, 0:3], start=False, stop=False)
                        nc.tensor.matmul(S[:, 4:8], ident[:], Xr[:, 3:7], start=False, stop=False)
                        nc.tensor.matmul(S[:, 0:4], ident[:], Xr[:, 1:5], start=False, stop=False)
                        nc.tensor.matmul(S[:, 4:7], ident[:], Xr[:, 5:8], start=False, stop=False)
                        # boundary j=0 (w-1) and j=7 (w+1) for both A and xn
                        nc.tensor.matmul(S[:, 0:1], subd[:], Ar[:, 7:8], start=False, stop=False)
                        nc.tensor.matmul(S[:, 0:1], subd[:], Xr[:, 7:8], start=False, stop=False)
                        nc.tensor.matmul(S[:, 7:8], supd[:], Ar[:, 0:1], start=False, stop=False)
                        nc.tensor.matmul(S[:, 7:8], supd[:], Xr[:, 0:1], start=False, stop=False)
                        # xn_r dh=0 last, full width, carries stop
                        nc.tensor.matmul(S[:, 0:4], ident[:], Xr[:, 0:4], start=False, stop=True)
                        nc.tensor.matmul(S[:, 4:8], ident[:], Xr[:, 4:8], start=False, stop=True)

                    nc.scalar.activation(ssb[:, 2 * i2 : 2 * i2 + 2], S2[:], ACTF.Copy)

                prod = prodpool.tile([128, R, NJ, C], F16, tag="prod")
                h = R // 2
                # split prod (and its first fold) per half-batch so the first
                # half overlaps the second half's PSUM evacuations
                nc.vector.tensor_mul(prod[:, 0:h], xc[:, 0:h], ssb[:, 0:h])
                pv_a = prod[:, 0:h].rearrange("p r j c -> p (r j) c")
                nc.vector.tensor_add(pv_a[:, :, 0:64], pv_a[:, :, 0:64], pv_a[:, :, 64:128])
                nc.vector.tensor_mul(prod[:, h:R], xc[:, h:R], ssb[:, h:R])
                pv_b = prod[:, h:R].rearrange("p r j c -> p (r j) c")
                nc.vector.tensor_add(pv_b[:, :, 0:64], pv_b[:, :, 0:64], pv_b[:, :, 64:128])
                # remaining folds over the whole batch
                v = prod[:].rearrange("p r j c -> p (r j) c")
                w = 64
                while w > 8:
                    hh = w // 2
                    nc.vector.tensor_add(v[:, :, 0:hh], v[:, :, 0:hh], v[:, :, hh:w])
                    w = hh
                dotr = statpool.tile([128, G], F32, tag="dotr")
                nc.vector.tensor_reduce(dotr[:], v[:, :, 0:w], AXIS.X, ALU.add)
                sim = statpool.tile([128, G], F32, tag="sim")
                nc.vector.tensor_scalar(sim[:], dotr[:], -1.0, None, ALU.add)
                nc.sync.dma_start(out_dram[ob], sim[:])

            for b in range(NB_IN):
                xb = xpool.tile([128, R, NJ, C], F16, tag="xb")
                nc.gpsimd.dma_start(
                    xb[:],
                    x_dram[b * R : (b + 1) * R].rearrange("r (p j) c -> p r j c", p=128),
                )
                sq = sqpool.tile([128, R, NJ, C], F16, tag="sq")
                nc.scalar.activation(sq[:], xb[:], ACTF.Square)
                ssr = statpool.tile([128, G], F32, tag="ssr")
                fold_reduce(ssr[:], sq)
                snorm = statpool.tile([128, G], F32, tag="snorm")
                nc.scalar.activation(snorm[:], ssr[:], ACTF.Sqrt, bias=eps[:])
                sinv = statpool.tile([128, G], F32, tag="sinv")
                nc.vector.reciprocal(sinv[:], snorm[:])
                invd = statpool.tile([128, G, 2], F16, tag="invd")
                nc.vector.tensor_scalar(invd[:, :, 0:1], sinv[:].unsqueeze(2), 1.0, None, ALU.mult)
                nc.vector.tensor_scalar(invd[:, :, 1:2], sinv[:].unsqueeze(2), 1.0, None, ALU.mult)

                xnb = xnpool.tile([128, R, NJ, C], F16, tag="xn")
                nc.vector.tensor_tensor(
                    xnb[:].rearrange("p r j (h two) -> p (r j) h two", two=2),
                    invd[:].unsqueeze(2).broadcast_to([128, G, C // 2, 2]),
                    xb[:].rearrange("p r j (h two) -> p (r j) h two", two=2),
                    ALU.mult,
                )
                xn_t[b] = xnb

                if b >= 2:
                    emit_output_batch(b - 2)

    nc.compile()
    return nc


def shard_inputs(input_image):
    """input_image [H, W, 1, C] f32 -> per-core in_maps (144 padded rows each)."""
    x = np.asarray(input_image).reshape(H, W, C).astype(np.float32, copy=False)
    ident, subd, supd = build_consts()
    in_maps = []
    for core in range(NCORES):
        lo = core * RPC
        shard = np.zeros((NB_IN * R, W, C), np.float32)
        # shard row i = global row (lo - 8 + i); valid range [lo-1, lo+128]
        gs = max(lo - 8, 0)
        ge = min(lo + RPC + 8, H)
        shard[gs - (lo - 8) : ge - (lo - 8)] = x[gs:ge]
        in_maps.append({"x": shard, "ident": ident, "subd": subd, "supd": supd})
    return in_maps


def unshard_output(results):
    """results[i]['out'] [NB_OUT, 128, G] -> [H, W] f32."""
    out = np.empty((H, W), np.float32)
    for core in range(NCORES):
        st = np.asarray(results[core]["out"]).reshape(NB_OUT, 128, R, NJ)
        sim = st.transpose(0, 2, 1, 3).reshape(RPC, W)  # w = 8p + j
        out[core * RPC : (core + 1) * RPC] = sim
    return out


_NC_CACHE = {}


def get_nc():
    if "nc" not in _NC_CACHE:
        _NC_CACHE["nc"] = build_bass()
    return _NC_CACHE["nc"]


def kernel(input_image):
    nc = get_nc()
    in_maps = shard_inputs(input_image)
    res = run_bass_kernel_spmd(nc, in_maps, list(range(NCORES)))
    return unshard_output(res.results)


if __name__ == "__main__":
    rng = np.random.default_rng(0)
    x = rng.standard_normal((H, W, 1, C), dtype=np.float32)
    out = kernel(x)
    print(out.shape, out.dtype, out[:2, :4])


# revision 9
# speedup vs baseline: 2.3471x; 1.0014x over previous
"""3x3 neighborhood cosine-similarity sum (minus self) on 8 TRN2 NeuronCores.

Input:  input_image [1024, 1024, 1, C=128] float32  (H, W, 1, C)
Output: sim [1024, 1024] float32

sim = <xn, BoxSum3x3(xn)> - 1, xn = x / max(||x||, eps) per pixel.

Sharding: H rows split 128/core across 8 cores; each core receives 144 rows
(its 128 + 8-row aligned halo padding, zeros outside the image).

Per-core layout: w = 8p + j  ->  SBUF tiles [128 p, R=8 rows, 8 j, 128 c].
Each partition line is 4KB-contiguous in HBM (fast DMA); horizontal w+-1 is
a free-dim shift except at j=0/7 (handled by sub/super-diagonal matmuls).

Engine split per row batch (R=8 rows):
 - cast-DMA (SWDGE)  : f32 HBM -> fp16 SBUF
 - ACT               : sq = Square(xb); sqrt(ss+eps); S~ PSUM->SBUF evac
 - DVE               : ss = tensor_reduce(sq); inv = 1/sqrt; dup inv pairs;
                       xn = inv-broadcast * xb (4D pair-broadcast TT at 2x);
                       A = xn_{r-1}+xn_{r+1}; prod = xn*S~;
                       dot = tensor_reduce(prod); sim = dot - 1
 - PE                : S~ = sum_{dh in -1,0,1} shift_dh(A_r) + shift_dh(xn_r)
                       via identity matmuls w/ shifted rhs APs accumulated in
                       PSUM + sub/super-diagonal boundary matmuls
"""

import numpy as np
import ml_dtypes

import sys

for _p in ("/opt/trn_rl_repo",):
    if _p not in sys.path:
        sys.path.insert(0, _p)

import concourse.bass as bass
import concourse.bacc as bacc
import concourse.mybir as mybir
import concourse.tile as tile
from concourse.bass_utils import run_bass_kernel_spmd

F32 = mybir.dt.float32
BF16 = mybir.dt.bfloat16
F16 = mybir.dt.float16
ALU = mybir.AluOpType
ACTF = mybir.ActivationFunctionType
AXIS = mybir.AxisListType

H, W, C = 1024, 1024, 128
NCORES = 8
RPC = H // NCORES          # 128 output rows per core
R = 8                      # rows per batch
NJ = 8                     # j per partition; w = 8p + j
NB_OUT = RPC // R          # 16 output batches
NB_IN = NB_OUT + 2         # 18 input batches = 144 rows (8-row halo pad each side)
G = R * NJ                 # 64 (row, j) groups per batch


def build_consts():
    ident = np.eye(128, dtype=np.float32)
    subd = np.zeros((128, 128), np.float32)
    supd = np.zeros((128, 128), np.float32)
    for p in range(127):
        subd[p, p + 1] = 1.0   # out[m] += rhs[m-1]
        supd[p + 1, p] = 1.0   # out[m] += rhs[m+1]
    bf = lambda a: a.astype(np.float16)
    return bf(ident), bf(subd), bf(supd)


def build_bass():
    nc = bacc.Bacc(None, target_bir_lowering=False)
    x_dram = nc.declare_dram_parameter("x", [NB_IN * R, W, C], F32, isOutput=False)
    id_dram = nc.declare_dram_parameter("ident", [128, 128], F16, isOutput=False)
    sub_dram = nc.declare_dram_parameter("subd", [128, 128], F16, isOutput=False)
    sup_dram = nc.declare_dram_parameter("supd", [128, 128], F16, isOutput=False)
    out_dram = nc.declare_dram_parameter("out", [NB_OUT, 128, G], F32, isOutput=True)

    with tile.TileContext(nc) as tc:
        with (
            tc.tile_pool(name="consts", bufs=1) as cpool,
            tc.tile_pool(name="xb", bufs=2) as xpool,
            tc.tile_pool(name="sq", bufs=1) as sqpool,
            tc.tile_pool(name="xn", bufs=4) as xnpool,
            tc.tile_pool(name="aa", bufs=2) as apool,
            tc.tile_pool(name="ssb", bufs=2) as ssbpool,
            tc.tile_pool(name="prod", bufs=1) as prodpool,
            tc.tile_pool(name="stat", bufs=2) as statpool,
            tc.tile_pool(name="psum", bufs=2, space="PSUM") as psumpool,
        ):
            ident = cpool.tile([128, 128], F16, tag="ident")
            subd = cpool.tile([128, 128], F16, tag="subd")
            supd = cpool.tile([128, 128], F16, tag="supd")
            nc.sync.dma_start(ident[:], id_dram[:])
            nc.sync.dma_start(subd[:], sub_dram[:])
            nc.sync.dma_start(supd[:], sup_dram[:])
            eps = cpool.tile([128, 1], F32, tag="eps")
            nc.gpsimd.memset(eps[:], 1e-4)

            xn_t = [None] * NB_IN

            def fold_reduce(out_f32, t4d):
                """Sum over c (last axis, 128 wide) of t4d [128, R, NJ, C] fp16
                into out_f32 [128, G] via in-place 2x fold cascade."""
                v = t4d[:].rearrange("p r j c -> p (r j) c")
                w = C
                while w > 8:
                    h = w // 2
                    nc.vector.tensor_add(v[:, :, 0:h], v[:, :, 0:h], v[:, :, h:w])
                    w = h
                nc.vector.tensor_reduce(out_f32, v[:, :, 0:w], AXIS.X, ALU.add)


            def emit_output_batch(ob):
                xp, xc, xx = xn_t[ob], xn_t[ob + 1], xn_t[ob + 2]
                xn_t[ob] = None

                # A = xn_{r-1} + xn_{r+1} (vertical neighbors)
                A = apool.tile([128, R, NJ, C], F16, tag="A")
                nc.vector.tensor_add(A[:, 0], xp[:, R - 1], xc[:, 1])
                nc.vector.tensor_add(A[:, 1 : R - 1], xc[:, 0 : R - 2], xc[:, 2:R])
                nc.vector.tensor_add(A[:, R - 1], xc[:, R - 2], xx[:, 0])

                ssb = ssbpool.tile([128, R, NJ, C], F16, tag="ssb")
                for i2 in range(R // 2):
                    S2 = psumpool.tile([128, 2, NJ, C], F32, tag="S")
                    for ii in range(2):
                        i = 2 * i2 + ii
                        S = S2[:, ii]
                        Ar = A[:, i]
                        Xr = xc[:, i]
                        # A: dh=0 (starts both banks), dh=-1, dh=+1, boundaries
                        nc.tensor.matmul(S[:, 0:4], ident[:], Ar[:, 0:4], start=True, stop=False)
                        nc.tensor.matmul(S[:, 4:8], ident[:], Ar[:, 4:8], start=True, stop=False)
                        nc.tensor.matmul(S[:, 1:4], ident[:], Ar[:, 0:3], start=False, stop=False)
                        nc.tensor.matmul(S[:, 4:8], ident[:], Ar[:, 3:7], start=False, stop=False)
                        nc.tensor.matmul(S[:, 0:4], ident[:], Ar[:, 1:5], start=False, stop=False)
                        nc.tensor.matmul(S[:, 4:7], ident[:], Ar[:, 5:8], start=False, stop=False)
                        # xn_r: dh=-1, dh=+1
                        nc.tensor.matmul(S[:, 1:4], ident[:], Xr[:, 0:3], start=False, stop=False)
                        nc.tensor.matmul(S[:, 4:8], ident[:], Xr[:, 3:7], start=False, stop=False)
                        nc.tensor.matmul(S[:, 0:4], ident[:], Xr[:, 1:5], start=False, stop=False)
                        nc.tensor.matmul(S[:, 4:7], ident[:], Xr[:, 5:8], start=False, stop=False)
                        # boundary j=0 (w-1) and j=7 (w+1) for both A and xn
                        nc.tensor.matmul(S[:, 0:1], subd[:], Ar[:, 7:8], start=False, stop=False)
                        nc.tensor.matmul(S[:, 0:1], subd[:], Xr[:, 7:8], start=False, stop=False)
                        nc.tensor.matmul(S[:, 7:8], supd[:], Ar[:, 0:1], start=False, stop=False)
                        nc.tensor.matmul(S[:, 7:8], supd[:], Xr[:, 0:1], start=False, stop=False)
                        # xn_r dh=0 last, full width, carries stop
                        nc.tensor.matmul(S[:, 0:4], ident[:], Xr[:, 0:4], start=False, stop=True)
                        nc.tensor.matmul(S[:, 4:8], ident[:], Xr[:, 4:8], start=False, stop=True)

                    nc.scalar.activation(ssb[:, 2 * i2 : 2 * i2 + 2], S2[:], ACTF.Copy)

                prod = prodpool.tile([128, R, NJ, C], F16, tag="prod")
                h = R // 2
                # split prod (and its first fold) per half-batch so the first
                # half overlaps the second half's PSUM evacuations
                nc.vector.tensor_mul(prod[:, 0:h], xc[:, 0:h], ssb[:, 0:h])
                pv_a = prod[:, 0:h].rearrange("p r j c -> p (r j) c")
                nc.vector.tensor_add(pv_a[:, :, 0:64], pv_a[:, :, 0:64], pv_a[:, :, 64:128])
                nc.vector.tensor_mul(prod[:, h:R], xc[:, h:R], ssb[:, h:R])
                pv_b = prod[:, h:R].rearrange("p r j c -> p (r j) c")
                nc.vector.tensor_add(pv_b[:, :, 0:64], pv_b[:, :, 0:64], pv_b[:, :, 64:128])
                # remaining folds over the whole batch
                v = prod[:].rearrange("p r j c -> p (r j) c")
                w = 64
                while w > 8:
                    hh = w // 2
                    nc.vector.tensor_add(v[:, :, 0:hh], v[:, :, 0:hh], v[:, :, hh:w])
                    w = hh
                dotr = statpool.tile([128, G], F32, tag="dotr")
                nc.vector.tensor_reduce(dotr[:], v[:, :, 0:w], AXIS.X, ALU.add)
                sim = statpool.tile([128, G], F32, tag="sim")
                nc.vector.tensor_scalar(sim[:], dotr[:], -1.0, None, ALU.add)
                nc.sync.dma_start(out_dram[ob], sim[:])

            for b in range(NB_IN):
                xb = xpool.tile([128, R, NJ, C], F16, tag="xb")
                nc.gpsimd.dma_start(
                    xb[:],
                    x_dram[b * R : (b + 1) * R].rearrange("r (p j) c -> p r j c", p=128),
                )
                sq = sqpool.tile([128, R, NJ, C], F16, tag="sq")
                nc.scalar.activation(sq[:], xb[:], ACTF.Square)
                ssr = statpool.tile([128, G], F32, tag="ssr")
                fold_reduce(ssr[:], sq)
                snorm = statpool.tile([128, G], F32, tag="snorm")
                nc.scalar.activation(snorm[:], ssr[:], ACTF.Sqrt, bias=eps[:])
                sinv = statpool.tile([128, G], F32, tag="sinv")
                nc.vector.reciprocal(sinv[:], snorm[:])
                invd = statpool.tile([128, G, 2], F16, tag="invd")
                nc.vector.tensor_scalar(invd[:, :, 0:1], sinv[:].unsqueeze(2), 1.0, None, ALU.mult)
                nc.vector.tensor_scalar(invd[:, :, 1:2], sinv[:].unsqueeze(2), 1.0, None, ALU.mult)

                xnb = xnpool.tile([128, R, NJ, C], F16, tag="xn")
                nc.vector.tensor_tensor(
                    xnb[:].rearrange("p r j (h two) -> p (r j) h two", two=2),
                    invd[:].unsqueeze(2).broadcast_to([128, G, C // 2, 2]),
                    xb[:].rearrange("p r j (h two) -> p (r j) h two", two=2),
                    ALU.mult,
                )
                xn_t[b] = xnb

                if b >= 2:
                    emit_output_batch(b - 2)

    nc.compile()
    return nc


def shard_inputs(input_image):
    """input_image [H, W, 1, C] f32 -> per-core in_maps (144 padded rows each)."""
    x = np.asarray(input_image).reshape(H, W, C).astype(np.float32, copy=False)
    ident, subd, supd = build_consts()
    in_maps = []
    for core in range(NCORES):
        lo = core * RPC
        shard = np.zeros((NB_IN * R, W, C), np.float32)
        # shard row i = global row (lo - 8 + i); valid range [lo-1, lo+128]
        gs = max(lo - 8, 0)
        ge = min(lo + RPC + 8, H)
        shard[gs - (lo - 8) : ge - (lo - 8)] = x[gs:ge]
        in_maps.append({"x": shard, "ident": ident, "subd": subd, "supd": supd})
    return in_maps


def unshard_output(results):
    """results[i]['out'] [NB_OUT, 128, G] -> [H, W] f32."""
    out = np.empty((H, W), np.float32)
    for core in range(NCORES):
        st = np.asarray(results[core]["out"]).reshape(NB_OUT, 128, R, NJ)
        sim = st.transpose(0, 2, 1, 3).reshape(RPC, W)  # w = 8p + j
        out[core * RPC : (core + 1) * RPC] = sim
    return out


_NC_CACHE = {}


def get_nc():
    if "nc" not in _NC_CACHE:
        _NC_CACHE["nc"] = build_bass()
    return _NC_CACHE["nc"]


def kernel(input_image):
    nc = get_nc()
    in_maps = shard_inputs(input_image)
    res = run_bass_kernel_spmd(nc, in_maps, list(range(NCORES)))
    return unshard_output(res.results)


if __name__ == "__main__":
    rng = np.random.default_rng(0)
    x = rng.standard_normal((H, W, 1, C), dtype=np.float32)
    out = kernel(x)
    print(out.shape, out.dtype, out[:2, :4])
